# revision 8
# baseline (speedup 1.0000x reference)
"""Trainium2 Bass kernel for nn_ContextualAttention (sparse_attention).

Contract: kernel(**inputs) takes FULL numpy inputs and returns the FULL
[2, 256, 48, 48] float32 output. Internally shards across 8 NeuronCores as
(batch b in {0,1}) x (side l/r) x (position-half in {0,1}).

Per-core device work for unit (b, side), half h:
  scores_T[p, l] = sum_{ki,kj,c} mid[c, y+ki-1, x+kj-1] * feat[c, ly+ki-1, lx+kj-1]
    (contraction tiled as 9 spatial offsets x 2 channel-halves of 128; the
     shifted windows are contiguous 1-D APs into 24-wide images; the x-edge
     wrap is handled by three host-sent variants with the contaminated
     column zeroed, selected by kj -- no patch tensors are materialized)
  scores_T *= inv_denom[l]  (host-computed feature-patch L2 norms)
  attn_T = softmax over l (free axis), scale 10
  attn   = transpose(attn_T)            (PE transpose, 128-blocks)
  out[cf, p] = sum_l rawT[l, cf] * attn[l, p]   (cf = c*16 + i*4 + j)

DMA strategy: every logical tensor is ONE contiguous-per-partition DMA
(host pre-packs [128, chunks*width] layouts) -- per-DMA fixed costs on
TRN2 are ~565ns SP issue + 625ns HWDGE + 900ns completion semaphore, so
instruction count dominates bytes for small transfers. Output is staged
in SBUF and shipped in a few grouped DMAs with shrinking sizes so the
final group's latency tail is short.

Host: downsample, build wrap-variants + rawT via as_strided, overlap-add the
transpose-conv contributions, cosine blend.
"""

import sys

for _p in ("/opt/trn_rl_repo", "/root/.axon_site/_ro/trn_rl_repo"):
    if _p not in sys.path:
        sys.path.append(_p)

import numpy as np
import ml_dtypes

BF16 = ml_dtypes.bfloat16

B, C, H, W = 2, 256, 48, 48
HD = WD = 24          # downsampled spatial
L = HD * WD           # 576 filter positions
PH = L // 2           # 288 positions per core (half)
CF = C * 16           # 4096 reconstruction features (c, i, j)
EPS_SUM = 2304 * 1e-4  # sum_k (f^2 + eps) = sumsq + K*eps
SCALE = 10.0
MHW = 14 * 24 + 2     # mh row length incl 1-elem zero guards
FPW = 26 * 24 + 2     # fp row length incl guards

# out-DMA group sizes over the 32 cf-tiles (decreasing => short tail)
OGROUPS = [10, 10, 6, 3, 2, 1]

_CACHED = {}


def _build_nc(reps=1):
    from concourse import bacc, mybir
    from concourse.dt import dt
    from concourse.tile import TileContext

    f32 = dt.float32
    f32r = dt.float32r
    bf16 = dt.bfloat16

    nc = bacc.Bacc("TRN2", target_bir_lowering=False, debug=False,
                   num_devices=8)
    mh_d = nc.declare_dram_parameter("mh3", [128, 6 * MHW], bf16,
                                     isOutput=False)
    fp_d = nc.declare_dram_parameter("fp3", [128, 6 * FPW], bf16,
                                     isOutput=False)
    rawa_d = nc.declare_dram_parameter("rawTa", [128, 4 * CF], bf16,
                                       isOutput=False)
    rawb_d = nc.declare_dram_parameter("rawTb", [64, CF], bf16,
                                       isOutput=False)
    id_d = nc.declare_dram_parameter("ident", [128, 128], bf16, isOutput=False)
    iv_d = nc.declare_dram_parameter("invd", [1, L], f32, isOutput=False)
    out_d = nc.declare_dram_parameter("out", [128, 32 * PH], f32,
                                      isOutput=True)

    AX = mybir.AxisListType.X
    OP = mybir.AluOpType
    AF = mybir.ActivationFunctionType

    # l-tiles for the 576-long filter axis: 4x128 + 64
    LT = [(0, 128), (128, 128), (256, 128), (384, 128), (512, 64)]

    with TileContext(nc) as tc:
        with (
            tc.tile_pool(name="persist", bufs=2) as pp,
            tc.tile_pool(name="ostage", bufs=1) as opool,
            tc.tile_pool(name="stats", bufs=4) as sp,
            tc.tile_pool(name="ps_score", bufs=2, space="PSUM") as ps_s,
            tc.tile_pool(name="ps_tr", bufs=2, space="PSUM") as ps_t,
            tc.tile_pool(name="ps_out", bufs=4, space="PSUM") as ps_o,
        ):
          for _rep in range(reps):
              # ---- persistent SBUF tensors + input DMAs ----
              mha = pp.tile([128, 6 * MHW], bf16, tag="mha", name="mha")
              fpa = pp.tile([128, 6 * FPW], bf16, tag="fpa", name="fpa")
              rawa = pp.tile([128, 4 * CF], bf16, tag="rawa", name="rawa")
              rawb = pp.tile([64, CF], bf16, tag="rawb", name="rawb")
              ident = pp.tile([128, 128], bf16, tag="ident", name="ident")
              attnT = [pp.tile([96, L], f32, tag=f"attnT{i}", name=f"attnT{i}")
                       for i in range(3)]
              attnTb = [pp.tile([96, L], bf16, tag=f"attnTb{i}", name=f"attnTb{i}")
                        for i in range(3)]
              attn = [pp.tile([128, PH], bf16, tag=f"attn{i}", name=f"attn{i}")
                      for i in range(5)]
              dinv = pp.tile([128, L], f32, tag="dinv", name="dinv")
              invd = pp.tile([1, L], f32, tag="invd", name="invd")
              ostage = opool.tile([128, 32 * PH], f32, tag="ostage",
                                  name="ostage")

              def mh(v, ch):
                  return mha[:, (ch * 3 + v) * MHW:(ch * 3 + v + 1) * MHW]

              def fp(v, ch):
                  return fpa[:, (ch * 3 + v) * FPW:(ch * 3 + v + 1) * FPW]

              def rawT(lt, lsz, cf):
                  if lt < 4:
                      return rawa[0:lsz, lt * CF + cf * 128:
                                   lt * CF + (cf + 1) * 128]
                  return rawb[0:lsz, cf * 128:(cf + 1) * 128]

              # issue order: small early tensors first, rawT last (needed
              # only at recon time ~20us in)
              nc.sync.dma_start(invd[:, :], iv_d[:, :])
              HM, HF = 3 * MHW, 3 * FPW
              nc.sync.dma_start(mha[:, 0:HM], mh_d[:, 0:HM])
              nc.sync.dma_start(fpa[:, 0:HF], fp_d[:, 0:HF])
              nc.sync.dma_start(mha[:, HM:], mh_d[:, HM:])
              nc.sync.dma_start(fpa[:, HF:], fp_d[:, HF:])
              nc.sync.dma_start(ident[:, :], id_d[:, :])
              nc.sync.dma_start(rawa[:, :], rawa_d[:, :])
              nc.sync.dma_start(rawb[:, :], rawb_d[:, :])
              # ---- broadcast inv_denom across partitions (Pool engine) ----
              nc.gpsimd.partition_broadcast(dinv[:, :], invd[0:1, :])

              # ---- scores + softmax, one 96-position tile at a time ----
              for t in range(3):
                  for lh in range(2):
                      ps = ps_s.tile([96, PH], f32, tag="ps", name="ps")
                      k = 0
                      for ch in range(2):
                          for ki in range(3):
                              for kj in range(3):
                                  lo = 1 + (4 * t + ki) * 24 + kj - 1
                                  ro = 1 + (12 * lh + ki) * 24 + kj - 1
                                  nc.tensor.matmul(
                                      ps[:, :],
                                      mh(kj, ch)[:, lo:lo + 96],
                                      fp(kj, ch)[:, ro:ro + PH],
                                      start=(k == 0), stop=(k == 17))
                                  k += 1
                      # normalize by feature-patch norms while leaving PSUM
                      nc.vector.tensor_mul(attnT[t][:, lh * PH:(lh + 1) * PH],
                                           ps[:, :],
                                           dinv[0:96, lh * PH:(lh + 1) * PH])
                  esum = sp.tile([96, 1], f32, tag="esum", name="esum")
                  rinv = sp.tile([96, 1], f32, tag="rinv", name="rinv")
                  nc.scalar.activation(attnT[t][:, :], attnT[t][:, :], AF.Exp,
                                       scale=SCALE, accum_out=esum[:, :])
                  nc.vector.reciprocal(rinv[:, :], esum[:, :])
                  # final 1/sum scale: halves on DVE+ACT in parallel
                  nc.vector.tensor_scalar_mul(attnTb[t][:, 0:PH],
                                              attnT[t][:, 0:PH], rinv[:, :])
                  nc.scalar.mul(attnTb[t][:, PH:], attnT[t][:, PH:],
                                rinv[:, :])

              # ---- transpose attn_T -> attn [l, p] ----
              def transposes(t):
                  for lt, (l0, lsz) in enumerate(LT):
                      tr = ps_t.tile([128, 96], bf16, tag="tr", name="tr")
                      nc.tensor.transpose(tr[0:lsz, :],
                                          attnTb[t][:, l0:l0 + lsz],
                                          ident[0:96, 0:96])
                      ceng = (nc.vector.tensor_copy, nc.scalar.copy)[lt % 2]
                      ceng(attn[lt][0:lsz, t * 96:(t + 1) * 96],
                           tr[0:lsz, :])

              transposes(0)
              transposes(1)

              # ---- reconstruction: out[cf, p] = sum_l rawT[l, cf] attn[l, p]
              # The first NE cf-tiles start on the t0/t1 position columns only
              # (ready before tile-2's softmax drains) to keep PE fed through
              # the last-tile softmax chain; their t2 columns finish after.
              NE = 4
              epo = []
              for cf in range(NE):
                  po = ps_o.tile([128, PH], f32, tag="po", name="po")
                  for lt, (l0, lsz) in enumerate(LT):
                      nc.tensor.matmul(
                          po[:, 0:192],
                          rawT(lt, lsz, cf),
                          attn[lt][0:lsz, 0:192],
                          start=(lt == 0), stop=(lt == 4))
                  epo.append(po)

              transposes(2)

              bounds = np.cumsum(OGROUPS).tolist()
              g0 = 0
              for cf in range(CF // 128):
                  if cf < NE:
                      po = epo[cf]
                      for lt, (l0, lsz) in enumerate(LT):
                          nc.tensor.matmul(
                              po[:, 192:PH],
                              rawT(lt, lsz, cf),
                              attn[lt][0:lsz, 192:PH],
                              start=(lt == 0), stop=(lt == 4))
                  else:
                      po = ps_o.tile([128, PH], f32, tag="po", name="po")
                      for lt, (l0, lsz) in enumerate(LT):
                          nc.tensor.matmul(
                              po[:, :],
                              rawT(lt, lsz, cf),
                              attn[lt][0:lsz, :],
                              start=(lt == 0), stop=(lt == 4))
                  dst = ostage[:, cf * PH:(cf + 1) * PH]
                  eng = (nc.vector.tensor_copy, nc.scalar.copy)[cf % 2]
                  eng(dst, po[:, :])
                  if cf + 1 in bounds:
                      nc.sync.dma_start(out_d[:, g0 * PH:(cf + 1) * PH],
                                        ostage[:, g0 * PH:(cf + 1) * PH])
                      g0 = cf + 1

    nc.compile()
    return nc


def _variants(img, rows):
    """img: [C, rows, 24] -> [3, C, rows*24+2] with 1-elem zero guards and the
    wrap-contaminated column zeroed per kj variant (kj=0: col 23, kj=2: col 0).
    """
    out = np.zeros((3, C, rows * 24 + 2), np.float32)
    vl = img.copy(); vl[:, :, 23] = 0.0
    vr = img.copy(); vr[:, :, 0] = 0.0
    for v, arr in enumerate((vl, img, vr)):
        out[v, :, 1:1 + rows * 24] = arr.reshape(C, rows * 24)
    return out


def _pack6(v3, width):
    """[3, C, width] -> [128, 6*width] with chunk order (ch, v), bf16."""
    return np.ascontiguousarray(
        v3.reshape(3, 2, 128, width).transpose(2, 1, 0, 3)
    ).reshape(128, -1).astype(BF16)


def _prep_inputs(inputs):
    """Build the 8 per-core input maps from the full problem inputs."""
    left = np.asarray(inputs["left"], dtype=np.float32)
    right = np.asarray(inputs["right"], dtype=np.float32)
    mid = np.asarray(inputs["mid"], dtype=np.float32)
    sl = np.asarray(inputs["shortcut_l"], dtype=np.float32)
    sr = np.asarray(inputs["shortcut_r"], dtype=np.float32)

    m_ds = mid[:, :, ::2, ::2]
    f_ds = [left[:, :, ::2, ::2], right[:, :, ::2, ::2]]

    # mh: rows y in [-1, 12] (h=0) / [11, 24] (h=1), zero at out-of-range
    mh3 = np.zeros((B, 2, 128, 6 * MHW), BF16)
    for b in range(B):
        for h in range(2):
            m14 = np.zeros((C, 14, 24), np.float32)
            if h == 0:
                m14[:, 1:14] = m_ds[b, :, 0:13]
            else:
                m14[:, 0:13] = m_ds[b, :, 11:24]
            mh3[b, h] = _pack6(_variants(m14, 14), MHW)
    # fp: rows y in [-1, 24]
    fp3 = np.zeros((B, 2, 128, 6 * FPW), BF16)
    invd = np.zeros((B, 2, 1, L), np.float32)
    for b in range(B):
        for side in range(2):
            f26 = np.zeros((C, 26, 24), np.float32)
            f26[:, 1:25] = f_ds[side][b]
            fp3[b, side] = _pack6(_variants(f26, 26), FPW)
            # host inv_denom: 3x3 window sums of per-pixel channel sumsq
            s = np.zeros((26, 26), np.float32)
            s[1:25, 1:25] = (f_ds[side][b] ** 2).sum(axis=0)
            d2 = np.zeros((24, 24), np.float32)
            for ki in range(3):
                for kj in range(3):
                    d2 += s[ki:ki + 24, kj:kj + 24]
            invd[b, side] = (1.0 / np.sqrt(d2 + EPS_SUM)).reshape(1, L)

    def raw_t(s):  # [C,48,48] -> [576, 4096] (l=(y,x), cf=(c,i,j))
        p = np.zeros((C, 50, 50), np.float32)
        p[:, 1:49, 1:49] = s
        st = p.strides
        v = np.lib.stride_tricks.as_strided(
            p, shape=(24, 24, C, 4, 4),
            strides=(2 * st[1], 2 * st[2], st[0], st[1], st[2]))
        return np.ascontiguousarray(v).reshape(L, CF)

    raws = [[raw_t(sl[b]), raw_t(sr[b])] for b in range(B)]
    ident = np.eye(128, dtype=np.float32)

    in_maps = []
    for core in range(8):
        b, side, h = core >> 2, (core >> 1) & 1, core & 1
        rw = raws[b][side].astype(BF16)
        in_maps.append({
            "mh3": mh3[b, h],
            "fp3": fp3[b, side],
            "rawTa": np.ascontiguousarray(
                rw[:512].reshape(4, 128, CF).transpose(1, 0, 2)
            ).reshape(128, 4 * CF),
            "rawTb": np.ascontiguousarray(rw[512:]),
            "ident": ident.astype(BF16),
            "invd": invd[b, side],
        })
    return in_maps


def _postprocess(results):
    """results: list of 8 dicts with 'out' [128, 32*288] -> full output."""
    y = np.zeros((B, 2, C, 48, 48), np.float32)
    for b in range(B):
        for side in range(2):
            feat = np.concatenate(
                [np.asarray(results[(b << 2) | (side << 1) | h]["out"])
                 .reshape(128, 32, PH).transpose(1, 0, 2).reshape(CF, PH)
                 for h in (0, 1)], axis=1)           # [4096, 576]
            contrib = feat.reshape(C, 4, 4, 24, 24)
            acc = np.zeros((C, 50, 50), np.float32)
            for i in range(4):
                for j in range(4):
                    acc[:, i:i + 48:2, j:j + 48:2] += contrib[:, i, j]
            y[b, side] = acc[:, 1:49, 1:49] * 0.25
    j = np.arange(W, dtype=np.float32)
    w = (0.5 * (np.cos(np.pi * j / (W - 1)) + 1.0)).reshape(1, 1, 1, W)
    return w * y[:, 0] + w[..., ::-1] * y[:, 1]


def _run(inputs, trace=False):
    from concourse.bass_utils import run_bass_kernel_spmd

    if "nc" not in _CACHED:
        _CACHED["nc"] = _build_nc()
    in_maps = _prep_inputs(inputs)
    res = run_bass_kernel_spmd(_CACHED["nc"], in_maps, list(range(8)),
                               trace=trace)
    return _postprocess(res.results), res


def kernel(**inputs):
    out, _ = _run(inputs)
    return out
